# revision 40
# baseline (speedup 1.0000x reference)
"""Trainium2 Bass kernel for nn_ConditionalMLN.

Math: the reference reduces exactly (cart.sum(-1) == 1 algebraically) to
    out = sum_r w_r * (G + cnt_r - S_r),   S_r = sum_g flag[r,g] * Z[r,g]
    Z = prod_k t_k,  t_k = select(mask_k, p[i_k], 1 - p[i_k])

Device strategy (R sharded over 8 cores, 2 rules each). Only flagged
groundings contribute to S_r, so the host COMPACTS each rule's flagged
groundings (~100K of 200K) into a fixed padded slot space — halving the
gather work vs. gathering all G groundings.

  * host builds a doubled bf16 table: table2[2i+m] = m ? p_i : 1-p_i,
    plus one zero row; pad slots point at the zero row (t=0 -> Z=0).
  * dma_gather (SWDGE row gather, 256B rows of 128 bf16) fetches the
    row containing each element's entry over 4 SWDGE queues with deep
    descriptor rings (48KB scratch) and 2 tiles in flight per queue
    (8 buffer slots, per-slot load semaphores — a shared counting
    semaphore would race since DMA completions are unordered across
    instructions). A tile's 3 plane-gathers go to 3 DIFFERENT queues
    ((t+k)%4) so a tile completes ~3x sooner, shortening the
    DVE/buffer-recycle feedback chain.
  * DVE extracts entry e via onehot(iota==e) multiply + windowed
    tensor_reduce into full-width ext planes; Z = ext0*ext1*ext2 and
    the per-rule segment reduce run once at the end over the full
    width (short back-to-back DVE ops read stale data: the next
    instruction's reads overtake the previous one's SBUF write-acks).
  * host: S_r = per-partition partials summed; out = w @ (G+cnt-S).
"""

import numpy as np
import ml_dtypes

R, G, K, N = 16, 200000, 3, 2000000
NCORES = 8
P = 128
RLOC = R // NCORES              # 2 rules per core

# --- compacted slot space (per rule) -----------------------------------
TCOLS = 16                      # columns per gather tile
CS = 800                        # columns per rule (50 tiles); 102400 slots
CAP = CS * P                    # slot capacity per rule (>= ~100K flagged)
COLS = RLOC * CS                # 1600 columns per core
NSLOT = COLS * P                # 204800 slots per core
NTILE = COLS // TCOLS           # 100 tiles (50 per rule)
TSLOT = TCOLS * P               # 2048 slots per tile

NROWS = 2 * N // P              # 31250 rows of 128 bf16 entries in table2
DEAD = 2 * N                    # flat index of the zero row (row NROWS, e=0)
NQ = 4                          # SWDGE queues
NBUF = 8                        # rows/idx buffers (2 tiles in flight/queue)

_CACHE = {}


def _build_program():
    from concourse import bass, mybir, library_config

    nc = bass.Bass("TRN2", target_bir_lowering=False, debug=False,
                   num_devices=NCORES, num_swdge_queues=NQ,
                   dynamic_dma_scratch_size=49152)

    f32, bf16, i16 = mybir.dt.float32, mybir.dt.bfloat16, mybir.dt.int16

    tbl_d = nc.declare_dram_parameter("tbl", [NROWS + 1, P], bf16,
                                      isOutput=False)
    # per-tile-major so each tile's idx load is one contiguous DMA (a
    # strided slice would be split into 2 DMACopy by the AP normalizer,
    # each inc'ing isem by 16, breaking the wait arithmetic below)
    idx_d = [nc.declare_dram_parameter(f"idx{k}", [NTILE * P, TCOLS * 8], i16,
                                       isOutput=False) for k in range(K)]
    e_d = [nc.declare_dram_parameter(f"e{k}", [P, COLS], bf16,
                                     isOutput=False) for k in range(K)]
    iota_d = nc.declare_dram_parameter("iota", [P, P], bf16, isOutput=False)
    y_d = nc.declare_dram_parameter("y", [P, RLOC], f32, isOutput=True)

    idx_s = [[nc.alloc_sbuf_tensor(f"idx{k}_{b}", [P, TCOLS * 8], i16)
              for b in range(NBUF)] for k in range(K)]
    rows_s = [[nc.alloc_sbuf_tensor(f"rows{k}_{b}", [P, TSLOT], bf16)
               for b in range(NBUF)] for k in range(K)]
    e_s = [nc.alloc_sbuf_tensor(f"e{k}_s", [P, COLS], bf16) for k in range(K)]
    iota_s = nc.alloc_sbuf_tensor("iota_s", [P, P], bf16)
    oh_s = nc.alloc_sbuf_tensor("oh_s", [P, TSLOT], bf16)
    ext_s = [nc.alloc_sbuf_tensor(f"ext{k}_s", [P, COLS], f32)
             for k in range(K)]
    acc_s = nc.alloc_sbuf_tensor("acc_s", [P, RLOC], f32)

    AluOp = mybir.AluOpType
    X = mybir.AxisListType.X

    # plane-spread queue map: gather (t, k) -> queue (t+k)%NQ; ordinal =
    # its 1-based position among all gathers emitted on that queue
    QOF = []
    _qc = [0] * NQ
    for _t in range(NTILE):
        row = []
        for _k in range(K):
            _q = (_t + _k) % NQ
            _qc[_q] += 1
            row.append((_q, _qc[_q]))
        QOF.append(row)

    with (
        nc.Block() as block,
        nc.semaphore("dsem") as dsem,     # e planes + iota loaded
        nc.semaphore("isem0") as isem0,   # idx loads per buffer slot
        nc.semaphore("isem1") as isem1,
        nc.semaphore("isem2") as isem2,
        nc.semaphore("isem3") as isem3,
        nc.semaphore("isem4") as isem4,
        nc.semaphore("isem5") as isem5,
        nc.semaphore("isem6") as isem6,
        nc.semaphore("isem7") as isem7,
        nc.semaphore("gsem0") as gsem0,   # gather completions per queue
        nc.semaphore("gsem1") as gsem1,
        nc.semaphore("gsem2") as gsem2,
        nc.semaphore("gsem3") as gsem3,
        nc.semaphore("vsem") as vsem,     # vector tile completions
        nc.semaphore("fsem") as fsem,     # final vector done
        nc.semaphore("osem") as osem,
    ):
        gsems = [gsem0, gsem1, gsem2, gsem3]
        isems = [isem0, isem1, isem2, isem3, isem4, isem5, isem6, isem7]

        @block.sync
        def _(sync):
            for k in range(K):
                sync.dma_start(out=e_s[k].ap(), in_=e_d[k][:]).then_inc(dsem, 16)
            sync.dma_start(out=iota_s.ap(), in_=iota_d[:]).then_inc(dsem, 16)
            for t in range(NTILE):
                if t >= NBUF:
                    # idx buf t%NBUF free once gathers of tile t-NBUF done
                    for k in range(K):
                        q, n = QOF[t - NBUF][k]
                        sync.wait_ge(gsems[q], 16 * n)
                for k in range(K):
                    sync.dma_start(
                        out=idx_s[k][t % NBUF].ap(),
                        in_=idx_d[k][t * P:(t + 1) * P, :],
                    ).then_inc(isems[t % NBUF], 16)
            sync.wait_ge(fsem, 1)
            sync.dma_start(out=y_d[:], in_=acc_s.ap()).then_inc(osem, 16)
            sync.wait_ge(osem, 16)

        @block.gpsimd
        def _(g):
            g.load_library(library_config.mlp)
            ns_reg = g.to_reg(TSLOT)
            for t in range(NTILE):
                g.wait_ge(isems[t % NBUF], 16 * K * (t // NBUF + 1))
                if t >= NBUF:
                    g.wait_ge(vsem, t - NBUF + 1)   # rows buf free
                for k in range(K):
                    q, _ = QOF[t][k]
                    out3 = rows_s[k][t % NBUF].ap().rearrange(
                        "p (c l) -> p c l", l=P)
                    g.dma_gather(
                        out_ap=out3,
                        in_ap=tbl_d[:],
                        idxs_ap=idx_s[k][t % NBUF].ap(),
                        num_idxs=TSLOT,
                        num_idxs_reg=ns_reg,
                        elem_size=P,
                        single_packet=False,
                        queue_num=q,
                    ).then_inc(gsems[q], 16)

        @block.vector
        def _(v):
            v.wait_ge(dsem, 16 * (K + 1))
            for t in range(NTILE):
                c0 = t * TCOLS
                for k in range(K):
                    q, n = QOF[t][k]
                    v.wait_ge(gsems[q], 16 * n)
                iota_b = iota_s.ap().unsqueeze(1).broadcast_to([P, TCOLS, P])
                oh3 = oh_s.ap().rearrange("p (c l) -> p c l", l=P)
                for k in range(K):
                    e_b = e_s[k].ap()[:, c0:c0 + TCOLS].unsqueeze(2) \
                        .broadcast_to([P, TCOLS, P])
                    rows3 = rows_s[k][t % NBUF].ap().rearrange(
                        "p (c l) -> p c l", l=P)
                    v.tensor_tensor(out=oh3, in0=iota_b, in1=e_b,
                                    op=AluOp.is_equal)
                    v.tensor_tensor(out=rows3, in0=rows3, in1=oh3,
                                    op=AluOp.mult)
                    last = v.tensor_reduce(
                        ext_s[k].ap()[:, c0:c0 + TCOLS], rows3, X, AluOp.add)
                last.then_inc(vsem, 1)
            # Z = ext0*ext1*ext2 (into ext0), then per-rule segment reduce.
            # Done once over the full width: short (<~512-elem) DVE ops
            # racing back-to-back read stale data (engine write-ack lag).
            z = ext_s[0].ap()
            v.tensor_tensor(out=z, in0=z, in1=ext_s[1].ap(), op=AluOp.mult)
            v.tensor_tensor(out=z, in0=z, in1=ext_s[2].ap(), op=AluOp.mult)
            red = None
            for r in range(RLOC):
                red = v.tensor_reduce(
                    acc_s.ap()[:, r:r + 1],
                    z[:, r * CS:(r + 1) * CS], X, AluOp.add)
            red.then_inc(fsem, 1)

    from concourse.library_overlay import lower_extended_insts
    lower_extended_insts(nc)

    # Build-time guard: every semaphore's total increments must match the
    # wait arithmetic above (a split DMA would double-increment).
    expect = {"dsem": 16 * (K + 1), "osem": 16, "vsem": NTILE, "fsem": 1}
    for q in range(NQ):
        expect[f"gsem{q}"] = 16 * _qc[q]
    for b in range(NBUF):
        expect[f"isem{b}"] = 16 * K * len(range(b, NTILE, NBUF))
    got = {name: 0 for name in expect}
    for blk in nc.m.functions[0].blocks:
        for inst in blk.instructions:
            si = inst.sync_info
            if si is None:
                continue
            for u in (si.on_update or []):
                if u.ant_name in got and u.update_mode in (
                        "sem-add-imm", "sem-inc"):
                    got[u.ant_name] += u.update_value
    assert got == expect, f"sem count mismatch: {got} != {expect}"
    return nc


def _prep_core(idx2):
    """idx2: [RLOC, CAP, K] int64 flat table2 indices (mask folded, padded,
    row-sorted per rule).

    Returns (idx16 list per plane [P, NSLOT//16] i16, e_planes [P, COLS] bf16
    per plane). Slot j (within rule r) = r*CAP + j; slot s -> [partition
    s%128, col s//128] to match dma_gather output layout.
    """
    flat = idx2.reshape(RLOC * CAP, K)          # slot-ordered [NSLOT, K]
    idx16s, e_planes = [], []
    for k in range(K):
        col = flat[:, k]
        row = (col >> 7).astype(np.int16)
        e = (col & 127).astype(np.float32).astype(ml_dtypes.bfloat16)
        # e-plane: slot s -> [partition s%128, col s//128]
        e_planes.append(np.ascontiguousarray(e.reshape(COLS, P).T))
        # idx16: wrapped-16 layout replicated across the 8 q7 core groups,
        # tile-major: [NTILE*P, TCOLS*8] with tile t at rows [t*P,(t+1)*P)
        w = row.reshape(NSLOT // 16, 16).T      # [16, NSLOT//16]
        w128 = np.tile(w, (8, 1))               # [128, NSLOT//16]
        wt = w128.reshape(P, NTILE, TCOLS * 8).transpose(1, 0, 2) \
            .reshape(NTILE * P, TCOLS * 8)
        idx16s.append(np.ascontiguousarray(wt))
    return idx16s, e_planes


def prepare_in_maps(posterior_prob, latent_var_inds, latent_neg_mask,
                    obs_zero_flag):
    # table2[2i]=1-p_i, table2[2i+1]=p_i, plus a zero row at the end
    p = np.asarray(posterior_prob).astype(np.float32)
    t2 = np.empty((NROWS + 1) * P, dtype=ml_dtypes.bfloat16)
    t2[0:2 * N:2] = (1.0 - p).astype(ml_dtypes.bfloat16)
    t2[1:2 * N:2] = p.astype(ml_dtypes.bfloat16)
    t2[2 * N:] = ml_dtypes.bfloat16(0.0)
    tbl = t2.reshape(NROWS + 1, P)

    iota = np.tile(np.arange(P, dtype=np.float32), (P, 1)) \
        .astype(ml_dtypes.bfloat16)

    inds = np.asarray(latent_var_inds).astype(np.int64)
    mask = np.asarray(latent_neg_mask).astype(np.int64)
    flag = np.asarray(obs_zero_flag)
    idx2_all = 2 * inds + mask                  # [R, G, K]

    in_maps = []
    for c in range(NCORES):
        idx2 = np.full((RLOC, CAP, K), DEAD, dtype=np.int64)
        for rl in range(RLOC):
            r = RLOC * c + rl
            sel = np.flatnonzero(flag[r])       # flagged groundings
            nf = sel.size
            assert nf <= CAP, f"rule {r}: {nf} flagged > capacity {CAP}"
            idx2[rl, :nf, :] = idx2_all[r, sel, :]
        idx16s, e_planes = _prep_core(idx2)
        m = {"tbl": tbl, "iota": iota}
        for k in range(K):
            m[f"idx{k}"] = idx16s[k]
            m[f"e{k}"] = e_planes[k]
        in_maps.append(m)
    return in_maps


def kernel(posterior_prob, observed_rule_cnts, rule_weights,
           latent_var_inds, latent_neg_mask, obs_zero_flag):
    observed_rule_cnts = np.asarray(observed_rule_cnts)
    rule_weights = np.asarray(rule_weights)

    if "nc" not in _CACHE:
        _CACHE["nc"] = _build_program()
    nc = _CACHE["nc"]

    in_maps = prepare_in_maps(posterior_prob, latent_var_inds,
                              latent_neg_mask, obs_zero_flag)

    from concourse.bass_utils import run_bass_kernel_spmd
    res = run_bass_kernel_spmd(nc, in_maps, core_ids=list(range(NCORES)))

    s = np.empty(R, dtype=np.float64)
    for c in range(NCORES):
        part = res.results[c]["y"].astype(np.float64)   # [P, RLOC]
        for rl in range(RLOC):
            s[RLOC * c + rl] = part[:, rl].sum()
    scores = np.float64(G) + observed_rule_cnts.astype(np.float64) - s
    out = rule_weights.astype(np.float64) @ scores
    return np.asarray([out], dtype=np.float32)
